# revision 28
# baseline (speedup 1.0000x reference)
"""Trainium2 Bass kernel for nn_CustomLoss (cross-entropy + worst-class masked loss).

Computes: loss = mean_i(logsumexp(output_i) - output_i[target_i])
          result = loss * (1 + mean_i(target_i in {3,5,8,9}))

Data-parallel over 8 NeuronCores: each core streams its 32768x1000 f32 shard
as 4 MB chunks at the HBM-read roofline. Rows are pre-sorted by target on the
host (the loss is permutation-invariant), so each [128,1000] tile's targets
fall inside a narrow per-tile column window computed from the data; the
target-logit gather is then a cheap windowed DVE scalar_tensor_tensor
((iota==t)*x row-reduce over ~32 columns) instead of a full-width pass.
Row-sums of exp are split within each chunk between the ACT accumulator
(NA tiles) and one multi-row ACT exp + DVE 3D tensor_reduce (G-NA tiles),
keeping both compute engines ~15% under the DMA streaming pace so the DMA
rings stay the pacer. The first and last chunks are split into sub-DMAs to
shorten pipeline ramp and drain; the worst-class count runs up front off the
target vector alone. Host sums the per-core partials ([128,2] each).
"""
import numpy as np
from contextlib import ExitStack

import concourse.bacc as bacc
import concourse.tile as tile
from concourse import mybir
from concourse.bass_utils import run_bass_kernel_spmd

F32 = mybir.dt.float32
AF = mybir.ActivationFunctionType
ALU = mybir.AluOpType

N_CORES = 8
B, C = 262144, 1000
ROWS = B // N_CORES           # 32768 rows per core
P = 128                       # SBUF partitions
G = 8                         # [128, C] sub-tiles per DMA chunk
N_CHUNKS = ROWS // (P * G)    # 32 chunks of [128, G, C] (4 MB contiguous)
N_TILES = ROWS // P           # 256 logical [128, C] tiles
NA = 3                        # ACT-accum tiles per chunk; G-NA via DVE reduce
BUFS_X = 3                    # in-flight full chunks (12 MB window)
WORST = (3.0, 5.0, 8.0, 9.0)

_CACHE = {}


def _build(los, w):
    """Compile the SPMD kernel for per-tile gather windows [los[k], los[k]+w)."""
    nc = bacc.Bacc(None, target_bir_lowering=False, debug=False,
                   num_devices=N_CORES)
    x_h = nc.declare_dram_parameter("x", [N_CHUNKS, P, G, C], F32, isOutput=False)
    tgt_h = nc.declare_dram_parameter("tgt", [P, N_TILES], F32, isOutput=False)
    iota_h = nc.declare_dram_parameter("iota", [P, C], F32, isOutput=False)
    out_h = nc.declare_dram_parameter("out", [P, 2], F32, isOutput=True)

    with tile.TileContext(nc) as tc, ExitStack() as ctx:
        xp = ctx.enter_context(tc.tile_pool(name="xp", bufs=BUFS_X))
        xh = ctx.enter_context(tc.tile_pool(name="xh", bufs=3))
        scr = ctx.enter_context(tc.tile_pool(name="scr", bufs=2))
        em = ctx.enter_context(tc.tile_pool(name="em", bufs=2))
        pers = ctx.enter_context(tc.tile_pool(name="pers", bufs=1))

        s_cols = pers.tile([P, N_TILES], F32, tag="s_cols")   # sum_j exp(x_ij)
        g_cols = pers.tile([P, N_TILES], F32, tag="g_cols")   # x_i[t_i]
        tgt_sb = pers.tile([P, N_TILES], F32, tag="tgt_sb")
        iota_sb = pers.tile([P, C], F32, tag="iota_sb")
        fin = pers.tile([P, 8], F32, tag="fin")
        out_sb = pers.tile([P, 2], F32, tag="out_sb")

        def gather(x_t, j, k):
            lo = los[k]
            m_scr = scr.tile([P, w], F32, tag="m_scr")
            nc.vector.scalar_tensor_tensor(
                out=m_scr[:], in0=iota_sb[:, lo:lo + w],
                scalar=tgt_sb[:, k:k + 1], in1=x_t[:, j, lo:lo + w],
                op0=ALU.is_equal, op1=ALU.mult,
                accum_out=g_cols[:, k:k + 1],
            )

        def act_accum(x_t, j, k):
            e_scr = scr.tile([P, C], F32, tag="e_scr")
            nc.scalar.activation(
                out=e_scr[:], in_=x_t[:, j, :], func=AF.Exp,
                accum_out=s_cols[:, k:k + 1],
            )

        def split_chunk(ch, subs):
            """Process chunk ch as sub-DMAs of the given tile counts,
            all tiles via ACT accum."""
            a = 0
            for sub in subs:
                x_t = xh.tile([P, sub, C], F32, tag="xh")
                nc.sync.dma_start(out=x_t[:], in_=x_h[ch][:, a:a + sub, :])
                for j in range(sub):
                    k = ch * G + a + j
                    act_accum(x_t, j, k)
                    gather(x_t, j, k)
                a += sub

        # First chunk as sub-DMAs so compute ramps immediately.
        split_chunk(0, [2, 2, 2, 2])

        nc.sync.dma_start(out=tgt_sb[:], in_=tgt_h[:])
        nc.sync.dma_start(out=iota_sb[:], in_=iota_h[:])

        # Worst-class count depends only on tgt: do it up front.
        eq = pers.tile([P, N_TILES], F32, tag="eq")
        nc.vector.tensor_scalar(
            out=eq[:], in0=tgt_sb[:], scalar1=WORST[0], scalar2=None,
            op0=ALU.is_equal,
        )
        for v in WORST[1:-1]:
            nc.vector.scalar_tensor_tensor(
                out=eq[:], in0=tgt_sb[:], scalar=v, in1=eq[:],
                op0=ALU.is_equal, op1=ALU.add,
            )
        nc.vector.scalar_tensor_tensor(
            out=eq[:], in0=tgt_sb[:], scalar=WORST[-1], in1=eq[:],
            op0=ALU.is_equal, op1=ALU.add,
            accum_out=out_sb[:, 1:2],
        )

        nm = G - NA
        for ch in range(1, N_CHUNKS - 1):
            x_t = xp.tile([P, G, C], F32, tag="x_t")
            nc.sync.dma_start(out=x_t[:], in_=x_h[ch])
            k0 = ch * G
            # multi-row exp for tiles [0, nm) + one DVE reduce
            e_mul = em.tile([P, nm, C], F32, tag="e_mul")
            nc.scalar.activation(
                out=e_mul[:], in_=x_t[:, 0:nm, :], func=AF.Exp,
            )
            nc.vector.tensor_reduce(
                out=s_cols[:, k0:k0 + nm], in_=e_mul[:],
                axis=mybir.AxisListType.X, op=ALU.add,
            )
            for j in range(nm, G):
                act_accum(x_t, j, k0 + j)
            for j in range(G):
                gather(x_t, j, k0 + j)

        # Tail chunk: the drain is ACT-paced, so the first two sub-chunks
        # route their row-sums to the idle DVE (exp-only + reduce); the last
        # two single-tile subs stay ACT-accum, keeping the post-stream chain
        # shortest (exp+accum -> read_acc -> Ln table load -> Ln). A DVE
        # reduce on the last tile would sit in front of the table load.
        chl = N_CHUNKS - 1
        a = 0
        for sub in (2, 2, 2):
            x_t = xh.tile([P, sub, C], F32, tag="xh")
            nc.sync.dma_start(out=x_t[:], in_=x_h[chl][:, a:a + sub, :])
            kq = chl * G + a
            e_tl = em.tile([P, sub, C], F32, tag="e_tl")
            nc.scalar.activation(out=e_tl[:], in_=x_t[:], func=AF.Exp)
            nc.vector.tensor_reduce(
                out=s_cols[:, kq:kq + sub], in_=e_tl[:],
                axis=mybir.AxisListType.X, op=ALU.add,
            )
            for j in range(sub):
                gather(x_t, j, kq + j)
            a += sub
        for sub in (1, 1):
            # own tag: these must not wait on the 2-tile subs' slot releases,
            # which recycle through the DVE reduce chain (~3.4us stream gap)
            x_t = xh.tile([P, sub, C], F32, tag="xh1")
            nc.sync.dma_start(out=x_t[:], in_=x_h[chl][:, a:a + sub, :])
            k = chl * G + a
            act_accum(x_t, 0, k)
            gather(x_t, 0, k)
            a += sub
        k_last = N_CHUNKS * G - 1

        # fin0 = sum_k ln(s_k); fin1 = sum_k x_t,k. The bulk of the g_cols
        # reduction runs before the last gather lands; only the last column
        # is folded in on the critical tail.
        nc.vector.tensor_reduce(
            out=fin[:, 2:3], in_=g_cols[:, 0:k_last],
            axis=mybir.AxisListType.X, op=ALU.add,
        )
        lse_cols = pers.tile([P, N_TILES], F32, tag="lse_cols")
        nc.scalar.activation(
            out=lse_cols[:], in_=s_cols[:], func=AF.Ln,
            accum_out=fin[:, 0:1],
        )
        nc.vector.tensor_tensor(
            out=fin[:, 1:2], in0=fin[:, 2:3], in1=g_cols[:, k_last:k_last + 1],
            op=ALU.add,
        )
        nc.vector.tensor_tensor(
            out=out_sb[:, 0:1], in0=fin[:, 0:1], in1=fin[:, 1:2], op=ALU.subtract,
        )

        nc.sync.dma_start(out=out_h[:], in_=out_sb[:])

    nc.compile()
    return nc


def _shard_inputs(output: np.ndarray, target: np.ndarray):
    """Sort rows by target per core, derive per-tile gather windows, repack.

    Returns (in_maps, los, w): tile k holds sorted rows [128k, 128k+128);
    los[k] is the window start shared by all cores, w the compiled width.
    """
    iota = np.tile(np.arange(C, dtype=np.float32), (P, 1))
    per_core = []
    lo_t = np.full(N_TILES, C, dtype=np.int64)
    hi_t = np.zeros(N_TILES, dtype=np.int64)
    for c in range(N_CORES):
        xs = output[c * ROWS:(c + 1) * ROWS]
        ts = target[c * ROWS:(c + 1) * ROWS]
        order = np.argsort(ts, kind="stable")
        xs = xs[order]
        ts = ts[order].astype(np.float32)
        t_tiles = ts.reshape(N_TILES, P)          # tile k = sorted rows 128k..
        lo_t = np.minimum(lo_t, t_tiles.min(axis=1).astype(np.int64))
        hi_t = np.maximum(hi_t, t_tiles.max(axis=1).astype(np.int64))
        per_core.append((xs, t_tiles))
    w = max(32, int((hi_t - lo_t).max()) + 1)
    los = [int(v) for v in np.minimum(lo_t, C - w)]

    in_maps = []
    for xs, t_tiles in per_core:
        # layout: tile k=ch*G+j, partition p holds sorted row 128*k + p,
        # i.e. x[ch, p, j] = xs[ch*(G*P) + j*P + p]
        in_maps.append({
            "x": np.ascontiguousarray(
                xs.reshape(N_CHUNKS, G, P, C).transpose(0, 2, 1, 3)),
            "tgt": np.ascontiguousarray(t_tiles.T),   # [P, N_TILES]
            "iota": iota,
        })
    return in_maps, los, w


def _combine(results) -> np.float32:
    nll = 0.0
    cnt = 0.0
    for r in results:
        nll += float(r["out"][:, 0].astype(np.float64).sum())
        cnt += float(r["out"][:, 1].astype(np.float64).sum())
    loss = nll / B
    mask_mean = cnt / B
    return np.float32(loss * (1.0 + mask_mean))


def _run(in_maps, los, w, **kwargs):
    key = (tuple(los), w)
    if _CACHE.get("key") != key:
        _CACHE["nc"] = _build(los, w)
        _CACHE["key"] = key
    return run_bass_kernel_spmd(_CACHE["nc"], in_maps, list(range(N_CORES)),
                                **kwargs)


def kernel(output: np.ndarray, target: np.ndarray) -> np.float32:
    assert output.shape == (B, C) and target.shape == (B,)
    in_maps, los, w = _shard_inputs(output, target)
    res = _run(in_maps, los, w)
    return _combine(res.results)
